# revision 6
# baseline (speedup 1.0000x reference)
"""Multi-head attention (B=4, S=2048, E=1024, H=16, D=64) on 8 trn2 cores.

Sharding: core c handles batch b=c//2 and head-group hg=c%2 (8 heads, 512
embed cols). QKV weights are column-sharded by head group so attention is
fully local per device.

Per-core kernel plan (all matmuls in float32r = full-rate fp32):
  phase A: DMA X[b] naturally, PE-transpose into XT (e on partitions).
           QT[d,q] = (Wq.T @ X.T) + bq   (d on partitions, scale folded later)
           KT[d,q] likewise; V[s,d] = X@Wv + bv (bias via K=1 ones matmul),
           stored augmented as [V | 1] per head for the Z row.
  phase B: per head-pair, per q-chunk(512):
           sT[k,q] = KT.T @ QT (two K=64 matmuls row-tiled at partitions 0/64)
           e = exp(0.125*sT)  (single ACT pass, PSUM->SBUF)
           ctxT_aug[65,q] += Vaug.T @ e  (row 64 = sum_k e = softmax denom Z)
           PE-transpose ctxT_aug 128-chunks -> [q,65], normalize by 1/Z (DVE),
           assemble [q, 512] output, DMA out.
"""

import numpy as np
from contextlib import ExitStack

import concourse.bass as bass
import concourse.mybir as mybir
import concourse.tile as tile
from concourse.bass import ts, ds
from concourse.masks import make_identity

B, S, E = 4, 2048, 1024
H, DH = 16, 64
NCORES = 8
HG = 2                # head groups per batch (cores per batch element)
HPC = H // HG         # heads per core = 8
CE = HPC * DH         # embed cols per core = 512
P = 128
NQT = S // P          # 16 q-tiles of 128
QC = 4                # q chunks of 512
KT_TILES = S // P     # 16 key tiles of 128
ET = E // P           # 8 e-tiles
MT = CE // P          # 4 output dim tiles (head pairs)

F32 = mybir.dt.float32
F32R = mybir.dt.float32r
AF = mybir.ActivationFunctionType


def _r(ap):
    return ap.bitcast(F32R)


def _build(tc, out, hs, wq, bq, wk, bk, wv, bv):
    nc = tc.nc
    with ExitStack() as ctx:
        persist = ctx.enter_context(tc.tile_pool(name="persist", bufs=1))

        # persistent buffers
        qt = [persist.tile([P, S], F32R, tag=f"qt{m}", name=f"qt{m}") for m in range(MT)]
        kt = [persist.tile([P, S], F32R, tag=f"kt{m}", name=f"kt{m}") for m in range(MT)]
        # V augmented: per s-tile [128, 8 heads, 64+1]; col 64 = ones (Z row)
        v = [persist.tile([P, HPC, DH + 1], F32R, tag=f"v{st}", name=f"v{st}") for st in range(NQT)]
        ident = persist.tile([P, P], F32, tag="ident")
        make_identity(nc, ident)
        bqs = persist.tile([P, MT], F32, tag="bqs")
        bks = persist.tile([P, MT], F32, tag="bks")
        nc.sync.dma_start(bqs, bq.rearrange("(o p) -> p o", p=P))
        nc.sync.dma_start(bks, bk.rearrange("(o p) -> p o", p=P))
        bvrow = persist.tile([1, CE], F32R, tag="bvrow")
        nc.sync.dma_start(bvrow, _r(bv[None, :]))
        ones_row_f32 = persist.tile([1, P], F32, tag="ones_row_f32")
        nc.vector.memset(ones_row_f32, 1.0)
        ones_row = persist.tile([1, P], F32R, tag="ones_row")
        nc.vector.tensor_copy(out=ones_row, in_=ones_row_f32)
        ones_col_f32 = persist.tile([P, HPC], F32, tag="ones_col_f32")
        nc.vector.memset(ones_col_f32, 1.0)

        hsr = hs.rearrange("(t p) e -> p t e", p=P)  # [128, 16, 1024]

        with tc.tile_pool(name="xt_pool", bufs=1) as xtp:
            xt = [xtp.tile([P, S], F32R, tag=f"xt{e}", name=f"xt{e}") for e in range(ET)]

            # ---- phase A1: transpose X -> XT ----
            with (
                tc.tile_pool(name="xs_pool", bufs=5) as xsp,
                tc.tile_pool(name="tr_psum", bufs=3, space="PSUM") as trp,
            ):
                for qg in range(QC):
                    xss = []
                    for j in range(4):
                        xs_t = xsp.tile([P, E], F32, tag="xs", name="xs")
                        nc.sync.dma_start(xs_t, hsr[:, qg * 4 + j, :])
                        xss.append(xs_t)
                    for e in range(ET):
                        tp = trp.tile([P, 512], F32, tag="tps", name="tps")
                        for j in range(4):
                            nc.tensor.transpose(
                                tp[:, ts(j, P)], xss[j][:, ts(e, P)], ident
                            )
                        nc.vector.tensor_copy(out=xt[e][:, ts(qg, 512)], in_=tp)

            # ---- phase A2: projections ----
            with (
                tc.tile_pool(name="w_pool", bufs=1) as wp,
                tc.tile_pool(name="proj_psum", bufs=6, space="PSUM") as pjp,
            ):
                # V first (phase B consumes all of V before finishing pair 0)
                w_all = wp.tile([P, ET, CE], F32R, tag="wall", name="wall")
                nc.sync.dma_start(w_all, _r(wv.rearrange("(o p) c -> p o c", p=P)))
                for sg in range(4):
                    pss = [pjp.tile([P, 512], F32, tag="pps", name="pps") for _ in range(4)]
                    for e in range(ET):
                        for si in range(4):
                            st = sg * 4 + si
                            nc.tensor.matmul(
                                pss[si],
                                lhsT=xt[e][:, ts(st, P)],
                                rhs=w_all[:, e, :],
                                start=(e == 0),
                                stop=False,
                            )
                    for si in range(4):
                        st = sg * 4 + si
                        # bias via K=1 matmul: ones_row.T @ bvrow
                        nc.tensor.matmul(
                            pss[si],
                            lhsT=ones_row,
                            rhs=bvrow,
                            start=False,
                            stop=True,
                        )
                        nc.vector.tensor_copy(
                            out=v[st][:, :, 0:DH],
                            in_=pss[si].rearrange("p (h d) -> p h d", h=HPC),
                        )
                        nc.vector.tensor_copy(out=v[st][:, :, DH], in_=ones_col_f32)

                # Q then K: out[d, q] accumulation over e-tiles
                for which, wsrc, dstt, bias in (
                    ("q", wq, qt, bqs),
                    ("k", wk, kt, bks),
                ):
                    w_all = wp.tile([P, ET, CE], F32R, tag="wall", name="wall")
                    nc.sync.dma_start(w_all, _r(wsrc.rearrange("(o p) c -> p o c", p=P)))
                    for qc in range(QC):
                        pss = [pjp.tile([P, 512], F32, tag="pps", name="pps") for _ in range(MT)]
                        for e in range(ET):
                            for m in range(MT):
                                nc.tensor.matmul(
                                    pss[m],
                                    lhsT=w_all[:, e, ts(m, P)],
                                    rhs=xt[e][:, ts(qc, 512)],
                                    start=(e == 0),
                                    stop=(e == ET - 1),
                                )
                        for m in range(MT):
                            nc.vector.tensor_scalar_add(
                                dstt[m][:, ts(qc, 512)], pss[m], bias[:, ts(m, 1)]
                            )

        # ---- phase B: attention ----
        with (
            tc.tile_pool(name="out_pool", bufs=1) as outp,
            tc.tile_pool(name="e_pool", bufs=3) as ep,
            tc.tile_pool(name="c_pool", bufs=2) as cp,
            tc.tile_pool(name="r_pool", bufs=2) as rp,
            tc.tile_pool(name="s_psum", bufs=2, space="PSUM") as spp,
            tc.tile_pool(name="ctx_psum", bufs=2, space="PSUM") as cpp,
            tc.tile_pool(name="t_psum", bufs=2, space="PSUM") as tpp,
        ):
            out_sb = outp.tile([P, NQT, CE], F32, tag="out_sb")
            out_r = out.rearrange("(t p) c -> p t c", p=P)

            for pr in range(MT):
                hA, hB = 2 * pr, 2 * pr + 1
                for qc in range(QC):
                    ctxA = cpp.tile([DH + 1, 512], F32, tag="ctx", name="ctx")
                    ctxB = cpp.tile([DH + 1, 512], F32, tag="ctx", name="ctx")
                    pending = None
                    for kti in range(KT_TILES + 1):
                        if kti < KT_TILES:
                            sps = spp.tile([P, 1024], F32, tag="sps", name="sps")
                            # scores^T for the head pair: K=64 matmuls at
                            # partition bases 0 / 64 (row-tiled, concurrent)
                            nc.tensor.matmul(
                                sps[:, 0:512],
                                lhsT=kt[pr][0:DH, ts(kti, P)],
                                rhs=qt[pr][0:DH, ts(qc, 512)],
                                start=True,
                                stop=True,
                            )
                            nc.tensor.matmul(
                                sps[:, 512:1024],
                                lhsT=kt[pr][DH:P, ts(kti, P)],
                                rhs=qt[pr][DH:P, ts(qc, 512)],
                                start=True,
                                stop=True,
                            )
                            et = ep.tile([P, 1024], F32R, tag="expT", name="expT")
                            nc.scalar.activation(et, sps, AF.Exp, scale=0.125)
                        # ctx matmuls staggered one k-tile behind scores so the
                        # PE never stalls waiting on the exp
                        if pending is not None:
                            pk, pe = pending
                            nc.tensor.matmul(
                                ctxA,
                                lhsT=v[pk][:, hA, :],
                                rhs=pe[:, 0:512],
                                start=(pk == 0),
                                stop=(pk == KT_TILES - 1),
                            )
                            nc.tensor.matmul(
                                ctxB,
                                lhsT=v[pk][:, hB, :],
                                rhs=pe[:, 512:1024],
                                start=(pk == 0),
                                stop=(pk == KT_TILES - 1),
                            )
                        if kti < KT_TILES:
                            pending = (kti, et)

                    for ctx_ps, hl in ((ctxA, hA), (ctxB, hB)):
                        cs = cp.tile([DH + 1, 512], F32, tag="cs", name="cs")
                        nc.vector.tensor_copy(out=cs, in_=ctx_ps)
                        pst = tpp.tile([P, 4, 72], F32, tag="pst", name="pst")
                        for j in range(4):
                            nc.tensor.transpose(
                                pst[:, j, 0 : DH + 1],
                                cs[:, ts(j, P)],
                                ident[0 : DH + 1, 0 : DH + 1],
                            )
                        rz = rp.tile([P, 4], F32, tag="rz", name="rz")
                        nc.vector.reciprocal(rz, pst[:, :, DH])
                        for j in range(4):
                            nc.vector.tensor_scalar_mul(
                                out_sb[:, qc * 4 + j, ds(hl * DH, DH)],
                                pst[:, j, 0:DH],
                                rz[:, ts(j, 1)],
                            )

            for qt_i in range(NQT):
                nc.sync.dma_start(out_r[:, qt_i, :], out_sb[:, qt_i, :])


def build_program():
    from concourse import bacc

    nc = bacc.Bacc("TRN2", target_bir_lowering=False, debug=False)
    hs = nc.dram_tensor("hs", [S, E], F32, kind="ExternalInput").ap()
    wq = nc.dram_tensor("wq", [E, CE], F32, kind="ExternalInput").ap()
    bq = nc.dram_tensor("bq", [CE], F32, kind="ExternalInput").ap()
    wk = nc.dram_tensor("wk", [E, CE], F32, kind="ExternalInput").ap()
    bk = nc.dram_tensor("bk", [CE], F32, kind="ExternalInput").ap()
    wv = nc.dram_tensor("wv", [E, CE], F32, kind="ExternalInput").ap()
    bv = nc.dram_tensor("bv", [CE], F32, kind="ExternalInput").ap()
    out = nc.dram_tensor("out", [S, CE], F32, kind="ExternalOutput").ap()
    with tile.TileContext(nc) as tc:
        _build(tc, out, hs, wq, bq, wk, bk, wv, bv)
    nc.compile()
    return nc


def make_in_maps(inputs):
    """Slice full inputs into 8 per-core input maps."""
    hs = np.ascontiguousarray(np.asarray(inputs["hidden_states"], dtype=np.float32))
    ws = {k: np.asarray(inputs[k], dtype=np.float32) for k in
          ("Wq", "bq", "Wk", "bk", "Wv", "bv")}
    in_maps = []
    for core in range(NCORES):
        b, hg = core // HG, core % HG
        cols = slice(hg * CE, (hg + 1) * CE)
        in_maps.append({
            "hs": hs[b],
            "wq": np.ascontiguousarray(ws["Wq"][:, cols]),
            "bq": np.ascontiguousarray(ws["bq"][cols]),
            "wk": np.ascontiguousarray(ws["Wk"][:, cols]),
            "bk": np.ascontiguousarray(ws["bk"][cols]),
            "wv": np.ascontiguousarray(ws["Wv"][:, cols]),
            "bv": np.ascontiguousarray(ws["bv"][cols]),
        })
    return in_maps


def assemble(results):
    """Gather 8 per-core [S, CE] outputs into the full [B, S, E] output."""
    out = np.empty((B, S, E), dtype=np.float32)
    for core in range(NCORES):
        b, hg = core // HG, core % HG
        out[b, :, hg * CE : (hg + 1) * CE] = results[core]["out"]
    return out


_NC_CACHE = None


def kernel(**inputs):
    global _NC_CACHE
    from concourse.bass_utils import run_bass_kernel_spmd

    if _NC_CACHE is None:
        _NC_CACHE = build_program()
    res = run_bass_kernel_spmd(_NC_CACHE, make_in_maps(inputs),
                               core_ids=list(range(NCORES)))
    return assemble(res.results)
